# revision 13
# baseline (speedup 1.0000x reference)
"""Birman-Schwinger core: K[b] = diag(sqrt|V_b|) @ R_0 @ diag(sqrt|V_b|).

With g[b,u] = sqrt(|V[b,u]| + eps) / (1 + u) and d = u - v:

    K[b,u,v] = g[b,u] * g[b,v] * H(d)
    H(d) = -0.5*sign(d)*sin(2d) + 0.5j*sign(d)*cos(2d)

Angle-difference identities make each output tile a sign-masked rank-2
outer product:

    Re K = -0.5*sign(d) * (a_u c_v - b_u s_v)
    Im K = +0.5*sign(d) * (b_u c_v + a_u s_v)

with a_u = g_u sin 2u, b_u = g_u cos 2u, c_v = g_v cos 2v, s_v = g_v sin 2v.
So the TensorEngine produces whole interleaved re/im tiles as K=6 bf16
matmuls (hi/lo bf16 splits of the row/column factors give ~fp32 accuracy),
with the +-0.5*sign(d) folded into the per-row-block weights: columns left
of the diagonal use the +0.5 variant, right of it the -0.5 variant, and the
single 128x128 diagonal block is fixed up by one elementwise multiply with
a constant sign mask. PSUM is drained to fp16 in SBUF alternately by the
Scalar and Vector engines, then DMAed out. The kernel is HBM-store-bound:
the output leaves the device as interleaved fp16 pairs (host upcasts to
complex64), halving store traffic vs f32.

Sharding: 8 cores; core c handles batch b = c // 2 and column half
h = c % 2 (all 4096 rows x 2048 complex columns). Row blocks are processed
in the order (s + 16h) % 32 so that diagonal-band blocks occupy program
slots 0..15 on every core - the instruction stream is identical across
cores and only the weight data differs; the host un-permutes row blocks
during assembly.
"""

import numpy as np

B = 4
N = 4096
NCORES = 8
P = 128                  # SBUF partitions
NSLOT = N // P           # 32 row blocks per core
NLOC = N // 2            # complex columns per core (column half)
EPS = 1e-10
FW = 2 * NLOC            # f16 columns per block row (4096)
PS = 1024                # f32 columns per PSUM drain chunk (2 banks)

_PROGRAM_CACHE = {}


def _build_program():
    import concourse.bacc as bacc
    import concourse.mybir as mybir
    from concourse.tile import TileContext

    nc = bacc.Bacc("TRN2", target_bir_lowering=False, debug=False)
    lhs = nc.dram_tensor(
        "t_lhs", [32, NSLOT * 2 * P], mybir.dt.bfloat16, kind="ExternalInput"
    ).ap()
    rhs = nc.dram_tensor("t_rhs", [32, FW], mybir.dt.bfloat16, kind="ExternalInput").ap()
    mask = nc.dram_tensor(
        "t_mask", [P, 2 * P], mybir.dt.float16, kind="ExternalInput"
    ).ap()
    out = nc.dram_tensor(
        "t_out", [N, FW], mybir.dt.float16, kind="ExternalOutput"
    ).ap()
    mult = mybir.AluOpType.mult

    # Drain split: DVE (0.96 GHz) takes 60/128 of the PSUM->SBUF chunks;
    # ScalarE (1.2 GHz) takes the rest.
    DVE_SHARE = 60
    NCHUNK = FW // PS  # drain chunks per block (4)

    with TileContext(nc) as tc:
        with tc.tile_pool(name="const", bufs=1) as cpool:
            # The PE only reaches its 2.4 GHz p-state when the matmul
            # contraction spans all 128 partitions (measured: K<=64 streams
            # run at 1.2 GHz, K=128 at 2.4 GHz, zero rows included). So the
            # K=6 factor tables sit in rows 0-5 of 128-partition tiles and
            # rows 6-127 are zero-filled on-chip by the (otherwise idle)
            # GpSimd engine, keeping the HBM loads at the real 6-row size.
            lhs_sb = cpool.tile([P, NSLOT * 2 * P], mybir.dt.bfloat16)
            rhs_sb = cpool.tile([P, FW], mybir.dt.bfloat16)
            mask_sb = cpool.tile([P, 2 * P], mybir.dt.float16)
            # Small first-slot loads so the first matmuls start early,
            # then the bulk.
            nc.sync.dma_start(out=lhs_sb[0:32, 0 : 2 * P], in_=lhs[:, 0 : 2 * P])
            nc.sync.dma_start(out=rhs_sb[0:32, 0:512], in_=rhs[:, 0:512])
            nc.sync.dma_start(out=lhs_sb[0:32, 2 * P :], in_=lhs[:, 2 * P :])
            nc.sync.dma_start(out=rhs_sb[0:32, 512:], in_=rhs[:, 512:])
            nc.sync.dma_start(out=mask_sb[:, :], in_=mask[:, :])
            # Pad-row zero fills (32-partition aligned; rows 6-31 come from
            # the host load), emitted in consumption order: slot 0's
            # weights, then the rhs sweep, then the remaining slots' weights.
            def pad_zero(tile, c0, c1):
                for p0 in range(32, P, 32):
                    nc.gpsimd.memset(tile[p0 : p0 + 32, c0:c1], 0)

            pad_zero(lhs_sb, 0, 2 * P)
            for q0 in range(0, FW, 512):
                pad_zero(rhs_sb, q0, q0 + 512)
            for s_ in range(1, NSLOT):
                pad_zero(lhs_sb, 2 * s_ * P, 2 * (s_ + 1) * P)

            with (
                tc.tile_pool(name="work", bufs=6) as wpool,
                tc.tile_pool(name="psum", bufs=4, space="PSUM") as ppool,
            ):
                ci = 0  # drain chunk counter (for the DVE/ScalarE split)
                for s in range(NSLOT):
                    w = wpool.tile([P, FW], mybir.dt.float16)
                    banded = s < 16
                    band_chunk = s // 2  # 512-col chunk holding the band

                    def wvar(v):
                        o = (2 * s + v) * P
                        return lhs_sb[:, o : o + P]

                    for half in range(NCHUNK):
                        pt = ppool.tile([P, PS], mybir.dt.float32)
                        c_lo = PS * half
                        for c in range(PS // 512):
                            j0 = c_lo + 512 * c
                            cc = j0 // 512
                            o = j0 - c_lo
                            if not banded or cc != band_chunk:
                                # uniform region: +0.5 weights left of the
                                # diagonal (or the whole row for non-banded
                                # slots), -0.5 weights right of it.
                                v = 0 if (not banded or cc < band_chunk) else 1
                                nc.tensor.matmul(
                                    out=pt[:, o : o + 512],
                                    lhsT=wvar(v),
                                    rhs=rhs_sb[:, j0 : j0 + 512],
                                    start=True,
                                    stop=True,
                                )
                            else:
                                # chunk straddles the diagonal band: two
                                # 256-col matmuls. The band half uses the
                                # +0.5 weights and is sign-fixed below.
                                h0v = 0  # s even: band | s odd: left
                                h1v = 1 if s % 2 == 0 else 0  # right | band
                                nc.tensor.matmul(
                                    out=pt[:, o : o + 256],
                                    lhsT=wvar(h0v),
                                    rhs=rhs_sb[:, j0 : j0 + 256],
                                    start=True,
                                    stop=True,
                                )
                                nc.tensor.matmul(
                                    out=pt[:, o + 256 : o + 512],
                                    lhsT=wvar(h1v),
                                    rhs=rhs_sb[:, j0 + 256 : j0 + 512],
                                    start=True,
                                    stop=True,
                                )
                        # PSUM -> SBUF fp16 drain, split across engines.
                        take_dve = (ci * DVE_SHARE) // 128 != ((ci + 1) * DVE_SHARE) // 128
                        if take_dve:
                            nc.vector.tensor_copy(
                                out=w[:, c_lo : c_lo + PS], in_=pt[:, :]
                            )
                        else:
                            nc.scalar.copy(out=w[:, c_lo : c_lo + PS], in_=pt[:, :])
                        ci += 1
                    if banded:
                        b0 = 256 * s
                        nc.vector.tensor_tensor(
                            out=w[:, b0 : b0 + 256],
                            in0=w[:, b0 : b0 + 256],
                            in1=mask_sb[:, :],
                            op=mult,
                        )
                    nc.sync.dma_start(out=out[s * P : (s + 1) * P, :], in_=w[:, :])
    nc.compile()
    return nc


def _get_program():
    if "nc" not in _PROGRAM_CACHE:
        _PROGRAM_CACHE["nc"] = _build_program()
    return _PROGRAM_CACHE["nc"]


def _host_tables(V):
    """Per-core input arrays (bf16 hi/lo-split trig factor tables)."""
    import ml_dtypes

    bf16 = ml_dtypes.bfloat16

    def split2(x):
        hi = x.astype(bf16)
        lo = (x - hi.astype(np.float64)).astype(bf16)
        return hi, lo

    pos = np.arange(N, dtype=np.float64)
    g = np.sqrt(np.abs(V).astype(np.float64) + EPS) / (1.0 + pos)  # (B, N) f64
    sin2 = np.sin(2.0 * pos)
    cos2 = np.cos(2.0 * pos)

    p_ = np.arange(P, dtype=np.int64)[:, None]
    q_ = np.arange(P, dtype=np.int64)[None, :]
    sgn = np.sign(p_ - q_).astype(np.float16)
    mask = np.empty((P, 2 * P), dtype=np.float16)
    mask[:, 0::2] = sgn
    mask[:, 1::2] = sgn

    in_maps = []
    for core in range(NCORES):
        b, h = divmod(core, 2)
        # column factors for this core's half
        q = np.arange(NLOC, dtype=np.int64) + NLOC * h
        c0, c1 = split2(g[b, q] * cos2[q])
        s0, s1 = split2(g[b, q] * sin2[q])
        rhs = np.zeros((32, FW), dtype=bf16)
        rhs[0, 0::2] = -c0
        rhs[0, 1::2] = s0
        rhs[1, 0::2] = -c1
        rhs[1, 1::2] = s1
        rhs[2] = rhs[0]
        rhs[3, 0::2] = s0
        rhs[3, 1::2] = c0
        rhs[4, 0::2] = s1
        rhs[4, 1::2] = c1
        rhs[5] = rhs[3]

        lhs = np.zeros((32, NSLOT * 2 * P), dtype=bf16)
        for s in range(NSLOT):
            j = (s + 16 * h) % NSLOT
            u = 128 * j + np.arange(P, dtype=np.int64)
            a = g[b, u] * sin2[u]
            bb = g[b, u] * cos2[u]
            for var in range(2):
                if s < 16:
                    sigma = 1.0 if var == 0 else -1.0
                else:
                    sigma = 1.0 if h == 0 else -1.0
                A0, A1 = split2(0.5 * sigma * a)
                B0, B1 = split2(0.5 * sigma * bb)
                col = (2 * s + var) * P
                lhs[0, col : col + P] = A0
                lhs[1, col : col + P] = A0
                lhs[2, col : col + P] = A1
                lhs[3, col : col + P] = B0
                lhs[4, col : col + P] = B0
                lhs[5, col : col + P] = B1

        in_maps.append({"t_lhs": lhs, "t_rhs": rhs, "t_mask": mask})
    return in_maps


def _run(in_maps, trace=False, **kwargs):
    from concourse import bass_utils

    nc = _get_program()
    return bass_utils.run_bass_kernel_spmd(
        nc, in_maps, core_ids=list(range(NCORES)), trace=trace, **kwargs
    )


def kernel(V):
    V = np.asarray(V, dtype=np.float32)
    assert V.shape == (B, N), V.shape
    in_maps = _host_tables(V)
    res = _run(in_maps, trace=False)
    out = np.empty((B, N, N), dtype=np.complex64)
    slot = np.arange(NSLOT)
    for core in range(NCORES):
        b, h = divmod(core, 2)
        plane = np.asarray(res.results[core]["t_out"], dtype=np.float32).view(
            np.complex64
        )  # (4096, 2048), rows in slot order
        j = (slot + 16 * h) % NSLOT  # slot -> global row block
        dst = out[b, :, NLOC * h : NLOC * (h + 1)].reshape(NSLOT, P, NLOC)
        dst[j] = plane.reshape(NSLOT, P, NLOC)
    return out


# revision 14
# speedup vs baseline: 1.1036x; 1.1036x over previous
"""Birman-Schwinger core: K[b] = diag(sqrt|V_b|) @ R_0 @ diag(sqrt|V_b|).

With g[b,u] = sqrt(|V[b,u]| + eps) / (1 + u) and d = u - v:

    K[b,u,v] = g[b,u] * g[b,v] * H(d)
    H(d) = -0.5*sign(d)*sin(2d) + 0.5j*sign(d)*cos(2d)

Angle-difference identities make each output tile a sign-masked rank-2
outer product:

    Re K = -0.5*sign(d) * (a_u c_v - b_u s_v)
    Im K = +0.5*sign(d) * (b_u c_v + a_u s_v)

with a_u = g_u sin 2u, b_u = g_u cos 2u, c_v = g_v cos 2v, s_v = g_v sin 2v.
So the TensorEngine produces whole interleaved re/im tiles as K=6 bf16
matmuls (hi/lo bf16 splits of the row/column factors give ~fp32 accuracy),
with the +-0.5*sign(d) folded into the per-row-block weights: columns left
of the diagonal use the +0.5 variant, right of it the -0.5 variant, and the
single 128x128 diagonal block is fixed up by one elementwise multiply with
a constant sign mask. PSUM is drained to fp16 in SBUF alternately by the
Scalar and Vector engines, then DMAed out. The kernel is HBM-store-bound:
the output leaves the device as interleaved fp16 pairs (host upcasts to
complex64), halving store traffic vs f32.

Sharding: 8 cores; core c handles batch b = c // 2 and column half
h = c % 2 (all 4096 rows x 2048 complex columns). Row blocks are processed
in the order (s + 16h) % 32 so that diagonal-band blocks occupy program
slots 0..15 on every core - the instruction stream is identical across
cores and only the weight data differs; the host un-permutes row blocks
during assembly.
"""

import numpy as np

B = 4
N = 4096
NCORES = 8
P = 128                  # SBUF partitions
NSLOT = N // P           # 32 row blocks per core
NLOC = N // 2            # complex columns per core (column half)
EPS = 1e-10
FW = 2 * NLOC            # f16 columns per block row (4096)
PS = 1024                # f32 columns per PSUM drain chunk (2 banks)

_PROGRAM_CACHE = {}


def _build_program():
    import concourse.bacc as bacc
    import concourse.mybir as mybir
    from concourse.tile import TileContext

    nc = bacc.Bacc("TRN2", target_bir_lowering=False, debug=False)
    lhs = nc.dram_tensor(
        "t_lhs", [32, NSLOT * 2 * P], mybir.dt.bfloat16, kind="ExternalInput"
    ).ap()
    rhs = nc.dram_tensor("t_rhs", [32, FW], mybir.dt.bfloat16, kind="ExternalInput").ap()
    mask = nc.dram_tensor(
        "t_mask", [P, 2 * P], mybir.dt.float16, kind="ExternalInput"
    ).ap()
    out = nc.dram_tensor(
        "t_out", [N, FW], mybir.dt.float16, kind="ExternalOutput"
    ).ap()
    mult = mybir.AluOpType.mult

    # Drain split: DVE (0.96 GHz) takes 60/128 of the PSUM->SBUF chunks;
    # ScalarE (1.2 GHz) takes the rest.
    DVE_SHARE = 60
    NCHUNK = FW // PS  # drain chunks per block (4)

    with TileContext(nc) as tc:
        with tc.tile_pool(name="const", bufs=1) as cpool:
            # The PE only reaches its 2.4 GHz p-state when the matmul
            # contraction spans all 128 partitions (measured: K<=64 streams
            # run at 1.2 GHz, K=128 at 2.4 GHz, zero rows included). So the
            # K=6 factor tables sit in rows 0-5 of 128-partition tiles and
            # rows 6-127 are zero-filled on-chip by the (otherwise idle)
            # GpSimd engine, keeping the HBM loads at the real 6-row size.
            lhs_sb = cpool.tile([P, NSLOT * 2 * P], mybir.dt.bfloat16)
            rhs_sb = cpool.tile([P, FW], mybir.dt.bfloat16)
            mask_sb = cpool.tile([P, 2 * P], mybir.dt.float16)
            # Small first-slot loads so the first matmuls start early; the
            # bulk loads are emitted after the work pools open so nothing
            # downstream serializes behind them.
            nc.sync.dma_start(out=lhs_sb[0:32, 0 : 2 * P], in_=lhs[:, 0 : 2 * P])
            nc.sync.dma_start(out=rhs_sb[0:32, 0:512], in_=rhs[:, 0:512])

            # Pad-row zero fills (32-partition aligned; rows 6-31 come from
            # the host load), emitted in consumption order: slot 0's
            # weights, then the rhs sweep, then the remaining slots' weights.
            def pad_zero(tile, c0, c1):
                for p0 in range(32, P, 32):
                    nc.gpsimd.memset(tile[p0 : p0 + 32, c0:c1], 0)

            pad_zero(lhs_sb, 0, 2 * P)
            for q0 in range(0, FW, 512):
                pad_zero(rhs_sb, q0, q0 + 512)

            with (
                tc.tile_pool(name="work", bufs=6) as wpool,
                tc.tile_pool(name="psum", bufs=4, space="PSUM") as ppool,
            ):
                nc.sync.dma_start(out=lhs_sb[0:32, 2 * P :], in_=lhs[:, 2 * P :])
                nc.sync.dma_start(out=rhs_sb[0:32, 512:], in_=rhs[:, 512:])
                nc.sync.dma_start(out=mask_sb[:, :], in_=mask[:, :])
                for s_ in range(1, NSLOT):
                    pad_zero(lhs_sb, 2 * s_ * P, 2 * (s_ + 1) * P)
                ci = 0  # drain chunk counter (for the DVE/ScalarE split)
                for s in range(NSLOT):
                    w = wpool.tile([P, FW], mybir.dt.float16)
                    banded = s < 16
                    band_chunk = s // 2  # 512-col chunk holding the band

                    def wvar(v):
                        o = (2 * s + v) * P
                        return lhs_sb[:, o : o + P]

                    for half in range(NCHUNK):
                        pt = ppool.tile([P, PS], mybir.dt.float32)
                        c_lo = PS * half
                        for c in range(PS // 512):
                            j0 = c_lo + 512 * c
                            cc = j0 // 512
                            o = j0 - c_lo
                            if not banded or cc != band_chunk:
                                # uniform region: +0.5 weights left of the
                                # diagonal (or the whole row for non-banded
                                # slots), -0.5 weights right of it.
                                v = 0 if (not banded or cc < band_chunk) else 1
                                nc.tensor.matmul(
                                    out=pt[:, o : o + 512],
                                    lhsT=wvar(v),
                                    rhs=rhs_sb[:, j0 : j0 + 512],
                                    start=True,
                                    stop=True,
                                )
                            else:
                                # chunk straddles the diagonal band: two
                                # 256-col matmuls. The band half uses the
                                # +0.5 weights and is sign-fixed below.
                                h0v = 0  # s even: band | s odd: left
                                h1v = 1 if s % 2 == 0 else 0  # right | band
                                nc.tensor.matmul(
                                    out=pt[:, o : o + 256],
                                    lhsT=wvar(h0v),
                                    rhs=rhs_sb[:, j0 : j0 + 256],
                                    start=True,
                                    stop=True,
                                )
                                nc.tensor.matmul(
                                    out=pt[:, o + 256 : o + 512],
                                    lhsT=wvar(h1v),
                                    rhs=rhs_sb[:, j0 + 256 : j0 + 512],
                                    start=True,
                                    stop=True,
                                )
                        # PSUM -> SBUF fp16 drain, split across engines.
                        take_dve = (ci * DVE_SHARE) // 128 != ((ci + 1) * DVE_SHARE) // 128
                        if take_dve:
                            nc.vector.tensor_copy(
                                out=w[:, c_lo : c_lo + PS], in_=pt[:, :]
                            )
                        else:
                            nc.scalar.copy(out=w[:, c_lo : c_lo + PS], in_=pt[:, :])
                        ci += 1
                    if banded:
                        b0 = 256 * s
                        nc.vector.tensor_tensor(
                            out=w[:, b0 : b0 + 256],
                            in0=w[:, b0 : b0 + 256],
                            in1=mask_sb[:, :],
                            op=mult,
                        )
                    nc.sync.dma_start(out=out[s * P : (s + 1) * P, :], in_=w[:, :])
    nc.compile()
    return nc


def _get_program():
    if "nc" not in _PROGRAM_CACHE:
        _PROGRAM_CACHE["nc"] = _build_program()
    return _PROGRAM_CACHE["nc"]


def _host_tables(V):
    """Per-core input arrays (bf16 hi/lo-split trig factor tables)."""
    import ml_dtypes

    bf16 = ml_dtypes.bfloat16

    def split2(x):
        hi = x.astype(bf16)
        lo = (x - hi.astype(np.float64)).astype(bf16)
        return hi, lo

    pos = np.arange(N, dtype=np.float64)
    g = np.sqrt(np.abs(V).astype(np.float64) + EPS) / (1.0 + pos)  # (B, N) f64
    sin2 = np.sin(2.0 * pos)
    cos2 = np.cos(2.0 * pos)

    p_ = np.arange(P, dtype=np.int64)[:, None]
    q_ = np.arange(P, dtype=np.int64)[None, :]
    sgn = np.sign(p_ - q_).astype(np.float16)
    mask = np.empty((P, 2 * P), dtype=np.float16)
    mask[:, 0::2] = sgn
    mask[:, 1::2] = sgn

    in_maps = []
    for core in range(NCORES):
        b, h = divmod(core, 2)
        # column factors for this core's half
        q = np.arange(NLOC, dtype=np.int64) + NLOC * h
        c0, c1 = split2(g[b, q] * cos2[q])
        s0, s1 = split2(g[b, q] * sin2[q])
        rhs = np.zeros((32, FW), dtype=bf16)
        rhs[0, 0::2] = -c0
        rhs[0, 1::2] = s0
        rhs[1, 0::2] = -c1
        rhs[1, 1::2] = s1
        rhs[2] = rhs[0]
        rhs[3, 0::2] = s0
        rhs[3, 1::2] = c0
        rhs[4, 0::2] = s1
        rhs[4, 1::2] = c1
        rhs[5] = rhs[3]

        lhs = np.zeros((32, NSLOT * 2 * P), dtype=bf16)
        for s in range(NSLOT):
            j = (s + 16 * h) % NSLOT
            u = 128 * j + np.arange(P, dtype=np.int64)
            a = g[b, u] * sin2[u]
            bb = g[b, u] * cos2[u]
            for var in range(2):
                if s < 16:
                    sigma = 1.0 if var == 0 else -1.0
                else:
                    sigma = 1.0 if h == 0 else -1.0
                A0, A1 = split2(0.5 * sigma * a)
                B0, B1 = split2(0.5 * sigma * bb)
                col = (2 * s + var) * P
                lhs[0, col : col + P] = A0
                lhs[1, col : col + P] = A0
                lhs[2, col : col + P] = A1
                lhs[3, col : col + P] = B0
                lhs[4, col : col + P] = B0
                lhs[5, col : col + P] = B1

        in_maps.append({"t_lhs": lhs, "t_rhs": rhs, "t_mask": mask})
    return in_maps


def _run(in_maps, trace=False, **kwargs):
    from concourse import bass_utils

    nc = _get_program()
    return bass_utils.run_bass_kernel_spmd(
        nc, in_maps, core_ids=list(range(NCORES)), trace=trace, **kwargs
    )


def kernel(V):
    V = np.asarray(V, dtype=np.float32)
    assert V.shape == (B, N), V.shape
    in_maps = _host_tables(V)
    res = _run(in_maps, trace=False)
    out = np.empty((B, N, N), dtype=np.complex64)
    slot = np.arange(NSLOT)
    for core in range(NCORES):
        b, h = divmod(core, 2)
        plane = np.asarray(res.results[core]["t_out"], dtype=np.float32).view(
            np.complex64
        )  # (4096, 2048), rows in slot order
        j = (slot + 16 * h) % NSLOT  # slot -> global row block
        dst = out[b, :, NLOC * h : NLOC * (h + 1)].reshape(NSLOT, P, NLOC)
        dst[j] = plane.reshape(NSLOT, P, NLOC)
    return out
